# revision 8
# baseline (speedup 1.0000x reference)
"""MoD router kernel for 8 Trainium2 NeuronCores (v3).

Full inputs: x [4, 8192, 1024] f32, w_router [1024] f32, w_block [1024, 1024] f32.
out[b, l] = gelu_tanh(x[b, l] @ w_block) if l in topk(x[b] @ w_router, k=6144)
            else x[b, l]

Sharding: core c <- batch row c//2, contiguous half c%2 of L (4096 tokens).

Design (v1 baseline ~235us):
  - w_block cast to bf16 + prepacked [p][c][e] on HOST -> single 2 MiB DMA.
  - x streams through SBUF in f32 (3 chunk buffers, prefetched 2 chunks
    ahead); only the bf16 cast is resident (GEMM input + pass-through rows).
  - PE transposes eliminated: XBAR DMA transpose (dma_start transpose=True)
    produces xt [d, c, t] in SBUF off-engine. PE runs ONLY the 512 GEMM
    matmuls (~109us at full clock) and must never stall (p-state!).
  - engine-queue discipline (keep every queue's per-tile work under PE's
    3.4us/tile): ACT does ONLY cast(i+3) + gelu(i) (+1 score-reduce per
    chunk); SP dispatches loads, XBAR transposes (i+3 lookahead; each costs
    its SEQ ~1.6us of descriptor gen) and stores; gpsimd does 1/4 of score
    mults, the collective, search reduces, and the fixup scatters; DVE does
    3/4 of scores (batched per chunk) and the threshold search.
  - scores stay exact f32 (min top-k boundary gap ~2.3e-5).
  - output is bf16 (stores 8 MiB; host upcasts). Pass-through rows scattered
    from resident bf16 x (OOB-skip trick), per-tile, each only depending on
    its own store.
  - pairwise AllGather of f32 scores + 6-round 16-ary dyadic threshold
    search from +-8 (resolution 9.5e-7 << min gap).
"""
import sys

if "/opt/trn_rl_repo" not in sys.path:
    sys.path.insert(0, "/opt/trn_rl_repo")

from contextlib import ExitStack

import numpy as np
import ml_dtypes

import concourse.bass as bass
import concourse.tile as tile
from concourse import bacc, mybir
from concourse.bass_utils import run_bass_kernel_spmd
from concourse import bass_isa

dt = mybir.dt
AF = mybir.ActivationFunctionType
ALU = mybir.AluOpType

P = 128
B, L, D = 4, 8192, 1024
TLOC = L // 2          # tokens per core
NT = TLOC // P         # 32 t-tiles per core
DC = D // P            # 8 contraction chunks
K_SEL = int(L * 0.75)  # 6144
N_ROUNDS = 6           # 16*16^-6 = 9.5e-7 resolution from +-8
SCORE_BOUND = 8.0
LOAD_CHUNK = 4         # t-tiles per load DMA (2 MiB)
CA = 3                 # cast/transpose lookahead (tiles)

_cached = {}


def build_kernel():
    nc = bacc.Bacc("TRN2", target_bir_lowering=False, debug=False, num_devices=8)
    x_d = nc.dram_tensor("x", [TLOC, D], dt.float32, kind="ExternalInput")
    wr_d = nc.dram_tensor("w_router", [D], dt.float32, kind="ExternalInput")
    wb_d = nc.dram_tensor("w_block", [P, DC, D], dt.bfloat16,
                          kind="ExternalInput")
    out_d = nc.dram_tensor("out", [TLOC, D], dt.bfloat16, kind="ExternalOutput")
    sc_in = nc.dram_tensor("sc_in", [TLOC], dt.float32, kind="Internal")
    sc_out = nc.dram_tensor("sc_out", [L], dt.float32, kind="Internal")

    with tile.TileContext(nc) as tc, ExitStack() as ctx:
        const = ctx.enter_context(tc.tile_pool(name="const", bufs=1))
        xsp = ctx.enter_context(tc.tile_pool(name="xs", bufs=3))
        xbfp = ctx.enter_context(tc.tile_pool(name="xbf", bufs=1))
        xtp = ctx.enter_context(tc.tile_pool(name="xt", bufs=6))
        yp = ctx.enter_context(tc.tile_pool(name="y", bufs=4))
        smalls = ctx.enter_context(tc.tile_pool(name="smalls", bufs=1))
        gscr = ctx.enter_context(tc.tile_pool(name="gscr", bufs=2))
        psy = ctx.enter_context(tc.tile_pool(name="psy", bufs=4, space="PSUM"))

        # ---- weight loads (single DMAs, host-prepacked) ----
        w_sb = const.tile([P, DC, D], dt.bfloat16)
        nc.sync.dma_start(w_sb[:], wb_d.ap())
        wr_sb = const.tile([1, D], dt.float32)
        nc.sync.dma_start(wr_sb[:], wr_d.ap())

        ones_row = const.tile([1, P], dt.float32)
        nc.vector.memset(ones_row[:], 1.0)
        # broadcast w_router over all partitions via K=1 matmuls
        w_rep = const.tile([P, D], dt.float32)
        for h in range(2):
            sl = slice(h * 512, (h + 1) * 512)
            pm = psy.tile([P, D], dt.float32, tag="psy")
            nc.tensor.matmul(pm[:, :512], ones_row[:], wr_sb[:, sl],
                             start=True, stop=True)
            nc.vector.tensor_copy(w_rep[:, sl], pm[:, :512])

        # ---- score / search tiles ----
        scores_loc = smalls.tile([P, NT], dt.float32)
        scores_full = smalls.tile([P, 2 * NT], dt.float32)
        ge3 = smalls.tile([P, 15, 2 * NT], dt.float32)
        cnts = smalls.tile([P, 15], dt.float32)
        cnts_red = smalls.tile([P, 15], dt.float32)
        gk = smalls.tile([P, 15], dt.float32)
        tcand = smalls.tile([P, 15], dt.float32)
        jrow_i = smalls.tile([P, 15], dt.int32)
        jrow = smalls.tile([P, 15], dt.float32)
        lo = smalls.tile([P, 1], dt.float32)
        w16t = smalls.tile([P, 1], dt.float32)
        m = smalls.tile([P, 1], dt.float32)
        msel = smalls.tile([P, NT], dt.float32)
        pcol_i = smalls.tile([P, 1], dt.int32)
        pcol = smalls.tile([P, 1], dt.float32)
        offs_f = smalls.tile([P, NT], dt.float32)
        offs = smalls.tile([P, NT], dt.int32)
        mscr = smalls.tile([P, 3, D], dt.float32)
        ascr = smalls.tile([P, D], dt.float32)

        xbf_all = xbfp.tile([P, NT, D], dt.bfloat16)

        # ---- streaming main loop, software-pipelined ----
        chunks = {}

        def ensure_chunk(j):
            if j < 0 or j >= NT // LOAD_CHUNK or j in chunks:
                return chunks.get(j)
            xs = xsp.tile([P, LOAD_CHUNK, D], dt.float32, tag="xs")
            a = j * LOAD_CHUNK
            with nc.named_scope("load"):
                nc.sync.dma_start(
                    xs[:],
                    x_d.ap()[a * P:(a + LOAD_CHUNK) * P, :].rearrange(
                        "(c p) d -> p c d", p=P))
            chunks[j] = xs
            return xs

        def stage_front(t):
            """Per tile: (chunk-level loads+scores) + cast + XBAR transpose."""
            j = t // LOAD_CHUNK
            xs = ensure_chunk(j)
            i = t % LOAD_CHUNK
            if i == 0:
                # prefetch upcoming chunks before SP blocks on cast sems
                ensure_chunk(j + 1)
                ensure_chunk(j + 2)
                with nc.named_scope("scores"):
                    # DVE: batched mult+reduce for tiles 4j..4j+2
                    wbc = w_rep[:].rearrange("p (a d) -> p a d", a=1) \
                        .to_broadcast([P, 3, D])
                    nc.vector.tensor_tensor(out=mscr[:], in0=xs[:, 0:3, :],
                                            in1=wbc, op=ALU.mult)
                    nc.vector.reduce_sum(scores_loc[:, 4 * j:4 * j + 3],
                                         mscr[:], axis=mybir.AxisListType.X)
                    # gpsimd: mult for tile 4j+3 (ACT reduces it 3 tiles later)
                    tmpg = gscr.tile([P, D], dt.float32, tag="tmpg")
                    nc.gpsimd.tensor_tensor(out=tmpg[:], in0=xs[:, 3, :],
                                            in1=w_rep[:], op=ALU.mult)
                    chunks[("g", j)] = tmpg
            with nc.named_scope("cast"):
                nc.scalar.copy(xbf_all[:, t, :], xs[:, i, :])
            if i == 3:
                with nc.named_scope("scores"):
                    nc.scalar.activation(ascr[:], chunks[("g", j)][:], AF.Copy,
                                         accum_out=scores_loc[:, t:t + 1])
            xt = xtp.tile([P, DC, P], dt.bfloat16, tag="xt")
            with nc.named_scope("xpose"):
                nc.sync.dma_start(xt[:], xbf_all[:, t, :], transpose=True)
            return xt

        xts = {}
        for t in range(CA):
            xts[t] = stage_front(t)

        store_insts = []
        for i in range(NT):
            if i + CA < NT:
                xts[i + CA] = stage_front(i + CA)
            xt = xts.pop(i)
            py = psy.tile([P, D], dt.float32, tag="psy")
            with nc.named_scope("gemm"):
                for h in range(2):
                    for c in range(DC):
                        nc.tensor.matmul(
                            py[:, h * 512:(h + 1) * 512], xt[:, c, :],
                            w_sb[:, c, h * 512:(h + 1) * 512],
                            start=(c == 0), stop=(c == DC - 1))
            y = yp.tile([P, D], dt.bfloat16, tag="y")
            with nc.named_scope("gelu"):
                nc.scalar.activation(y[:], py[:], AF.Gelu_apprx_tanh)
            with nc.named_scope("store"):
                st = nc.sync.dma_start(out_d.ap()[i * P:(i + 1) * P, :], y[:])
            store_insts.append(st)

        # ---- threshold side-chain (high priority so it never starves) ----
        with tc.high_priority():
            with nc.named_scope("coll"):
                nc.sync.dma_start(sc_in.ap(), scores_loc[:])
                nc.gpsimd.collective_compute(
                    "AllGather", ALU.bypass,
                    ins=[sc_in.ap()], outs=[sc_out.ap()],
                    replica_groups=[[0, 1], [2, 3], [4, 5], [6, 7]])
                nc.sync.dma_start(scores_full[:], sc_out.ap())
            with nc.named_scope("search"):
                # jrow = 1..15 replicated on every partition
                nc.gpsimd.iota(jrow_i[:], pattern=[[1, 15]], base=1,
                               channel_multiplier=0)
                nc.vector.tensor_copy(out=jrow[:], in_=jrow_i[:])
                nc.vector.memset(lo[:], -SCORE_BOUND)
                nc.vector.memset(w16t[:], 2.0 * SCORE_BOUND / 16.0)
                sc_b = scores_full[:].rearrange("p (a x) -> p a x", a=1) \
                    .to_broadcast([P, 15, 2 * NT])
                t_b = tcand[:].rearrange("p (j x) -> p j x", x=1) \
                    .to_broadcast([P, 15, 2 * NT])
                for r in range(N_ROUNDS):
                    # tcand[:, j] = lo + (j+1)*w16  (dyadic, exact fp32)
                    nc.vector.tensor_scalar(out=tcand[:], in0=jrow[:],
                                            scalar1=w16t[:], scalar2=lo[:],
                                            op0=ALU.mult, op1=ALU.add)
                    nc.vector.tensor_tensor(out=ge3[:], in0=sc_b, in1=t_b,
                                            op=ALU.is_ge)
                    nc.vector.reduce_sum(cnts[:], ge3[:],
                                         axis=mybir.AxisListType.X)
                    nc.gpsimd.partition_all_reduce(
                        cnts_red[:], cnts[:], P, bass_isa.ReduceOp.add)
                    # gk = (count >= k); m = #intervals passed (row-sum)
                    nc.vector.tensor_scalar(out=gk[:], in0=cnts_red[:],
                                            scalar1=float(K_SEL), scalar2=None,
                                            op0=ALU.is_ge)
                    nc.vector.reduce_sum(m[:], gk[:],
                                         axis=mybir.AxisListType.X)
                    # lo += m*w16 (bit-identical to the compared grid point)
                    nc.vector.tensor_scalar(out=lo[:], in0=m[:],
                                            scalar1=w16t[:], scalar2=lo[:],
                                            op0=ALU.mult, op1=ALU.add)
                    nc.vector.tensor_scalar_mul(w16t[:], w16t[:], 1.0 / 16.0)
            with nc.named_scope("mask"):
                # selected = score >= thr(=lo); offs = p + sel*2^30 (per-tile)
                nc.vector.tensor_scalar(out=msel[:], in0=scores_loc[:],
                                        scalar1=lo[:], scalar2=None,
                                        op0=ALU.is_ge)
                nc.gpsimd.iota(pcol_i[:], pattern=[[0, 1]], base=0,
                               channel_multiplier=1)
                nc.vector.tensor_copy(out=pcol[:], in_=pcol_i[:])
                nc.vector.tensor_scalar(out=offs_f[:], in0=msel[:],
                                        scalar1=float(2 ** 30),
                                        scalar2=pcol[:],
                                        op0=ALU.mult, op1=ALU.add)
                nc.vector.tensor_copy(out=offs[:], in_=offs_f[:])

        # ---- fixup: overwrite pass-through rows with resident bf16 x rows ----
        with nc.named_scope("fixup"):
            for i in range(NT):
                sl = out_d.ap()[i * P:(i + 1) * P, :]
                sl_rel = bass.AP(tensor=sl.tensor, offset=0, ap=sl.ap,
                                 dep_tracking_offset=i * P * D)
                fx = nc.gpsimd.indirect_dma_start(
                    out=sl_rel,
                    out_offset=bass.IndirectOffsetOnAxis(ap=offs[:, i:i + 1],
                                                         axis=0),
                    in_=xbf_all[:, i, :],
                    in_offset=None,
                    element_offset=i * P * D,
                    bounds_check=P - 1,
                    oob_is_err=False,
                )
                tile.add_dep_helper(fx.ins, store_insts[i].ins,
                                    reason="fixup scatter after bulk y store")

    nc.compile()
    return nc


def _get_nc():
    if "nc" not in _cached:
        _cached["nc"] = build_kernel()
    return _cached["nc"]


def run(x, w_router, w_block, trace=False, trace_kwargs=None):
    nc = _get_nc()
    x = np.ascontiguousarray(x, dtype=np.float32)
    w_router = np.ascontiguousarray(w_router, dtype=np.float32)
    w_block = np.ascontiguousarray(w_block, dtype=np.float32)
    # host prepack: w_sb[p, c, e] = w_block[c*128+p, e], bf16
    w_bf = np.ascontiguousarray(
        w_block.astype(ml_dtypes.bfloat16).reshape(DC, P, D).transpose(1, 0, 2))
    in_maps = []
    for c in range(8):
        b, h = c // 2, c % 2
        in_maps.append({
            "x": x[b, h * TLOC:(h + 1) * TLOC, :],
            "w_router": w_router,
            "w_block": w_bf,
        })
    res = run_bass_kernel_spmd(nc, in_maps, core_ids=list(range(8)),
                               trace=trace, **(trace_kwargs or {}))
    out = np.empty((B, L, D), dtype=np.float32)
    for c in range(8):
        b, h = c // 2, c % 2
        out[b, h * TLOC:(h + 1) * TLOC, :] = \
            res.results[c]["out"].astype(np.float32)
    return out, res


def kernel(x, w_router, w_block):
    out, _ = run(x, w_router, w_block, trace=False)
    return out


# revision 10
# speedup vs baseline: 1.0576x; 1.0576x over previous
"""MoD router kernel for 8 Trainium2 NeuronCores (v3).

Full inputs: x [4, 8192, 1024] f32, w_router [1024] f32, w_block [1024, 1024] f32.
out[b, l] = gelu_tanh(x[b, l] @ w_block) if l in topk(x[b] @ w_router, k=6144)
            else x[b, l]

Sharding: core c <- batch row c//2, contiguous half c%2 of L (4096 tokens).

Design (v1 baseline ~235us):
  - w_block cast to bf16 + prepacked [p][c][e] on HOST -> single 2 MiB DMA.
  - x streams through SBUF in f32 (3 chunk buffers, prefetched 2 chunks
    ahead); only the bf16 cast is resident (GEMM input + pass-through rows).
  - PE transposes eliminated: XBAR DMA transpose (dma_start transpose=True)
    produces xt [d, c, t] in SBUF off-engine. PE runs ONLY the 512 GEMM
    matmuls (~109us at full clock) and must never stall (p-state!).
  - engine-queue discipline (keep every queue's per-tile work under PE's
    3.4us/tile): ACT does ONLY cast(i+3) + gelu(i) (+1 score-reduce per
    chunk); SP dispatches loads, XBAR transposes (i+3 lookahead; each costs
    its SEQ ~1.6us of descriptor gen) and stores; gpsimd does 1/4 of score
    mults, the collective, search reduces, and the fixup scatters; DVE does
    3/4 of scores (batched per chunk) and the threshold search.
  - scores stay exact f32 (min top-k boundary gap ~2.3e-5).
  - output is bf16 (stores 8 MiB; host upcasts). Pass-through rows scattered
    from resident bf16 x (OOB-skip trick), per-tile, each only depending on
    its own store.
  - pairwise AllGather of f32 scores + 6-round 16-ary dyadic threshold
    search from +-8 (resolution 9.5e-7 << min gap).
"""
import sys

if "/opt/trn_rl_repo" not in sys.path:
    sys.path.insert(0, "/opt/trn_rl_repo")

from contextlib import ExitStack

import numpy as np
import ml_dtypes

import concourse.bass as bass
import concourse.tile as tile
from concourse import bacc, mybir
from concourse.bass_utils import run_bass_kernel_spmd
from concourse import bass_isa

dt = mybir.dt
AF = mybir.ActivationFunctionType
ALU = mybir.AluOpType

P = 128
B, L, D = 4, 8192, 1024
TLOC = L // 2          # tokens per core
NT = TLOC // P         # 32 t-tiles per core
DC = D // P            # 8 contraction chunks
K_SEL = int(L * 0.75)  # 6144
N_ROUNDS = 6           # 16*16^-6 = 9.5e-7 resolution from +-8
SCORE_BOUND = 8.0
LOAD_CHUNK = 4         # t-tiles per load DMA (2 MiB)
CA = 3                 # cast/transpose lookahead (tiles)

_cached = {}


def build_kernel():
    nc = bacc.Bacc("TRN2", target_bir_lowering=False, debug=False, num_devices=8)
    x_d = nc.dram_tensor("x", [TLOC, D], dt.float32, kind="ExternalInput")
    wr_d = nc.dram_tensor("w_router", [D], dt.float32, kind="ExternalInput")
    wb_d = nc.dram_tensor("w_block", [P, DC, D], dt.bfloat16,
                          kind="ExternalInput")
    out_d = nc.dram_tensor("out", [TLOC, D], dt.bfloat16, kind="ExternalOutput")
    sc_in = nc.dram_tensor("sc_in", [TLOC], dt.float32, kind="Internal")
    sc_out = nc.dram_tensor("sc_out", [L], dt.float32, kind="Internal")

    with tile.TileContext(nc) as tc, ExitStack() as ctx:
        const = ctx.enter_context(tc.tile_pool(name="const", bufs=1))
        xsp = ctx.enter_context(tc.tile_pool(name="xs", bufs=3))
        xbfp = ctx.enter_context(tc.tile_pool(name="xbf", bufs=1))
        xtp = ctx.enter_context(tc.tile_pool(name="xt", bufs=8))
        yp = ctx.enter_context(tc.tile_pool(name="y", bufs=4))
        smalls = ctx.enter_context(tc.tile_pool(name="smalls", bufs=1))
        gscr = ctx.enter_context(tc.tile_pool(name="gscr", bufs=2))
        psy = ctx.enter_context(tc.tile_pool(name="psy", bufs=4, space="PSUM"))

        # ---- weight loads (single DMAs, host-prepacked) ----
        w_sb = const.tile([P, DC, D], dt.bfloat16)
        nc.sync.dma_start(w_sb[:], wb_d.ap())
        wr_sb = const.tile([1, D], dt.float32)
        nc.sync.dma_start(wr_sb[:], wr_d.ap())

        ones_row = const.tile([1, P], dt.float32)
        nc.vector.memset(ones_row[:], 1.0)
        # broadcast w_router over all partitions via K=1 matmuls
        w_rep = const.tile([P, D], dt.float32)
        for h in range(2):
            sl = slice(h * 512, (h + 1) * 512)
            pm = psy.tile([P, D], dt.float32, tag="psy")
            nc.tensor.matmul(pm[:, :512], ones_row[:], wr_sb[:, sl],
                             start=True, stop=True)
            nc.vector.tensor_copy(w_rep[:, sl], pm[:, :512])

        # ---- score / search tiles ----
        scores_loc = smalls.tile([P, NT], dt.float32)
        scores_full = smalls.tile([P, 2 * NT], dt.float32)
        ge3 = smalls.tile([P, 15, 2 * NT], dt.float32)
        cnts = smalls.tile([P, 15], dt.float32)
        cnts_red = smalls.tile([P, 15], dt.float32)
        gk = smalls.tile([P, 15], dt.float32)
        tcand = smalls.tile([P, 15], dt.float32)
        jrow_i = smalls.tile([P, 15], dt.int32)
        jrow = smalls.tile([P, 15], dt.float32)
        lo = smalls.tile([P, 1], dt.float32)
        w16t = smalls.tile([P, 1], dt.float32)
        m = smalls.tile([P, 1], dt.float32)
        msel = smalls.tile([P, NT], dt.float32)
        pcol_i = smalls.tile([P, 1], dt.int32)
        pcol = smalls.tile([P, 1], dt.float32)
        offs_f = smalls.tile([P, NT], dt.float32)
        offs = smalls.tile([P, NT], dt.int32)
        mscr = smalls.tile([P, 3, D], dt.float32)
        ascr = smalls.tile([P, D], dt.float32)

        xbf_all = xbfp.tile([P, NT, D], dt.bfloat16)

        # ---- streaming main loop, software-pipelined ----
        # Casts + scores race ahead at LOAD pace (xbf_all is resident, so no
        # pool limit); only the xt runway (bufs) ties the XBAR transposes to
        # PE progress. The cast-emission lookahead starts at 4 tiles and
        # grows by one per iteration to 13 so the ACT queue is never asked to
        # do more than ~2 casts + 1 gelu per PE tile.
        chunks = {}

        def ensure_chunk(j):
            if j < 0 or j >= NT // LOAD_CHUNK or j in chunks:
                return chunks.get(j)
            xs = xsp.tile([P, LOAD_CHUNK, D], dt.float32, tag="xs")
            a = j * LOAD_CHUNK
            with nc.named_scope("load"):
                if j == 0:
                    # split first chunk so tiles 0-1 land ~3us earlier
                    nc.sync.dma_start(
                        xs[:, 0:2, :],
                        x_d.ap()[a * P:(a + 2) * P, :].rearrange(
                            "(c p) d -> p c d", p=P))
                    nc.sync.dma_start(
                        xs[:, 2:4, :],
                        x_d.ap()[(a + 2) * P:(a + 4) * P, :].rearrange(
                            "(c p) d -> p c d", p=P))
                else:
                    nc.sync.dma_start(
                        xs[:],
                        x_d.ap()[a * P:(a + LOAD_CHUNK) * P, :].rearrange(
                            "(c p) d -> p c d", p=P))
            chunks[j] = xs
            return xs

        def emit_cast(t):
            """Cast tile t (+ chunk-level load prefetch and scores)."""
            j = t // LOAD_CHUNK
            xs = ensure_chunk(j)
            i = t % LOAD_CHUNK
            if i == 0:
                ensure_chunk(j + 1)
                ensure_chunk(j + 2)
                with nc.named_scope("scores"):
                    # DVE: batched mult+reduce for tiles 4j..4j+2
                    wbc = w_rep[:].rearrange("p (a d) -> p a d", a=1) \
                        .to_broadcast([P, 3, D])
                    nc.vector.tensor_tensor(out=mscr[:], in0=xs[:, 0:3, :],
                                            in1=wbc, op=ALU.mult)
                    nc.vector.reduce_sum(scores_loc[:, 4 * j:4 * j + 3],
                                         mscr[:], axis=mybir.AxisListType.X)
                    # gpsimd: mult for tile 4j+3 (ACT reduces it later)
                    tmpg = gscr.tile([P, D], dt.float32, tag="tmpg")
                    nc.gpsimd.tensor_tensor(out=tmpg[:], in0=xs[:, 3, :],
                                            in1=w_rep[:], op=ALU.mult)
                    chunks[("g", j)] = tmpg
            with nc.named_scope("cast"):
                nc.scalar.copy(xbf_all[:, t, :], xs[:, i, :])
            if i == 3:
                with nc.named_scope("scores"):
                    nc.scalar.activation(ascr[:], chunks[("g", j)][:], AF.Copy,
                                         accum_out=scores_loc[:, t:t + 1])

        def emit_xpose(t):
            xt = xtp.tile([P, DC, P], dt.bfloat16, tag="xt")
            with nc.named_scope("xpose"):
                nc.sync.dma_start(xt[:], xbf_all[:, t, :], transpose=True)
            return xt

        cast_done = 0
        xts = {}
        xp_done = 0

        def advance_front(i):
            nonlocal cast_done, xp_done
            cast_target = min(NT, min(6 + 2 * i, i + 13))
            while cast_done < cast_target:
                emit_cast(cast_done)
                cast_done += 1
            xp_target = min(NT, i + 8, cast_done)
            while xp_done < xp_target:
                xts[xp_done] = emit_xpose(xp_done)
                xp_done += 1

        for t in range(4):
            emit_cast(t)
        cast_done = 4
        advance_front(0)

        store_insts = []
        for i in range(NT):
            advance_front(i)
            xt = xts.pop(i)
            py = psy.tile([P, D], dt.float32, tag="psy")
            with nc.named_scope("gemm"):
                for h in range(2):
                    for c in range(DC):
                        nc.tensor.matmul(
                            py[:, h * 512:(h + 1) * 512], xt[:, c, :],
                            w_sb[:, c, h * 512:(h + 1) * 512],
                            start=(c == 0), stop=(c == DC - 1))
            y = yp.tile([P, D], dt.bfloat16, tag="y")
            with nc.named_scope("gelu"):
                nc.scalar.activation(y[:], py[:], AF.Gelu_apprx_tanh)
            with nc.named_scope("store"):
                st = nc.sync.dma_start(out_d.ap()[i * P:(i + 1) * P, :], y[:])
            store_insts.append(st)

        # ---- threshold side-chain (high priority so it never starves) ----
        with tc.high_priority():
            with nc.named_scope("coll"):
                nc.sync.dma_start(sc_in.ap(), scores_loc[:])
                nc.gpsimd.collective_compute(
                    "AllGather", ALU.bypass,
                    ins=[sc_in.ap()], outs=[sc_out.ap()],
                    replica_groups=[[0, 1], [2, 3], [4, 5], [6, 7]])
                nc.sync.dma_start(scores_full[:], sc_out.ap())
            with nc.named_scope("search"):
                # jrow = 1..15 replicated on every partition
                nc.gpsimd.iota(jrow_i[:], pattern=[[1, 15]], base=1,
                               channel_multiplier=0)
                nc.vector.tensor_copy(out=jrow[:], in_=jrow_i[:])
                nc.vector.memset(lo[:], -SCORE_BOUND)
                nc.vector.memset(w16t[:], 2.0 * SCORE_BOUND / 16.0)
                sc_b = scores_full[:].rearrange("p (a x) -> p a x", a=1) \
                    .to_broadcast([P, 15, 2 * NT])
                t_b = tcand[:].rearrange("p (j x) -> p j x", x=1) \
                    .to_broadcast([P, 15, 2 * NT])
                for r in range(N_ROUNDS):
                    # tcand[:, j] = lo + (j+1)*w16  (dyadic, exact fp32)
                    nc.vector.tensor_scalar(out=tcand[:], in0=jrow[:],
                                            scalar1=w16t[:], scalar2=lo[:],
                                            op0=ALU.mult, op1=ALU.add)
                    nc.vector.tensor_tensor(out=ge3[:], in0=sc_b, in1=t_b,
                                            op=ALU.is_ge)
                    nc.vector.reduce_sum(cnts[:], ge3[:],
                                         axis=mybir.AxisListType.X)
                    nc.gpsimd.partition_all_reduce(
                        cnts_red[:], cnts[:], P, bass_isa.ReduceOp.add)
                    # gk = (count >= k); m = #intervals passed (row-sum)
                    nc.vector.tensor_scalar(out=gk[:], in0=cnts_red[:],
                                            scalar1=float(K_SEL), scalar2=None,
                                            op0=ALU.is_ge)
                    nc.vector.reduce_sum(m[:], gk[:],
                                         axis=mybir.AxisListType.X)
                    # lo += m*w16 (bit-identical to the compared grid point)
                    nc.vector.tensor_scalar(out=lo[:], in0=m[:],
                                            scalar1=w16t[:], scalar2=lo[:],
                                            op0=ALU.mult, op1=ALU.add)
                    nc.vector.tensor_scalar_mul(w16t[:], w16t[:], 1.0 / 16.0)
            with nc.named_scope("mask"):
                # selected = score >= thr(=lo); offs = p + sel*2^30 (per-tile)
                nc.vector.tensor_scalar(out=msel[:], in0=scores_loc[:],
                                        scalar1=lo[:], scalar2=None,
                                        op0=ALU.is_ge)
                nc.gpsimd.iota(pcol_i[:], pattern=[[0, 1]], base=0,
                               channel_multiplier=1)
                nc.vector.tensor_copy(out=pcol[:], in_=pcol_i[:])
                nc.vector.tensor_scalar(out=offs_f[:], in0=msel[:],
                                        scalar1=float(2 ** 30),
                                        scalar2=pcol[:],
                                        op0=ALU.mult, op1=ALU.add)
                nc.vector.tensor_copy(out=offs[:], in_=offs_f[:])

        # ---- fixup: overwrite pass-through rows with resident bf16 x rows ----
        with nc.named_scope("fixup"):
            for i in range(NT):
                sl = out_d.ap()[i * P:(i + 1) * P, :]
                sl_rel = bass.AP(tensor=sl.tensor, offset=0, ap=sl.ap,
                                 dep_tracking_offset=i * P * D)
                fx = nc.gpsimd.indirect_dma_start(
                    out=sl_rel,
                    out_offset=bass.IndirectOffsetOnAxis(ap=offs[:, i:i + 1],
                                                         axis=0),
                    in_=xbf_all[:, i, :],
                    in_offset=None,
                    element_offset=i * P * D,
                    bounds_check=P - 1,
                    oob_is_err=False,
                )
                tile.add_dep_helper(fx.ins, store_insts[i].ins,
                                    reason="fixup scatter after bulk y store")

    nc.compile()
    return nc


def _get_nc():
    if "nc" not in _cached:
        _cached["nc"] = build_kernel()
    return _cached["nc"]


def run(x, w_router, w_block, trace=False, trace_kwargs=None):
    nc = _get_nc()
    x = np.ascontiguousarray(x, dtype=np.float32)
    w_router = np.ascontiguousarray(w_router, dtype=np.float32)
    w_block = np.ascontiguousarray(w_block, dtype=np.float32)
    # host prepack: w_sb[p, c, e] = w_block[c*128+p, e], bf16
    w_bf = np.ascontiguousarray(
        w_block.astype(ml_dtypes.bfloat16).reshape(DC, P, D).transpose(1, 0, 2))
    in_maps = []
    for c in range(8):
        b, h = c // 2, c % 2
        in_maps.append({
            "x": x[b, h * TLOC:(h + 1) * TLOC, :],
            "w_router": w_router,
            "w_block": w_bf,
        })
    res = run_bass_kernel_spmd(nc, in_maps, core_ids=list(range(8)),
                               trace=trace, **(trace_kwargs or {}))
    out = np.empty((B, L, D), dtype=np.float32)
    for c in range(8):
        b, h = c // 2, c % 2
        out[b, h * TLOC:(h + 1) * TLOC, :] = \
            res.results[c]["out"].astype(np.float32)
    return out, res


def kernel(x, w_router, w_block):
    out, _ = run(x, w_router, w_block, trace=False)
    return out
